# revision 22
# baseline (speedup 1.0000x reference)
"""Trainium2 Bass kernel for the DSS (Diagonal State Space) layer.

y = irfft(rfft(u, 2L) * rfft(K, 2L))[:L] + D*u, with K the length-L DSS kernel
derived from (Lambda, W, log_step) via a complex softmax.

Implementation: the FFT convolution is mathematically a causal conv with an
exponentially-structured kernel K[s] = Re(sum_n wt_n * r_n^s).  We evaluate it
as a chunked diagonal-SSM scan on-device:
  - time-major layout, chunks of C=256 timesteps (2 partition tiles of 128)
  - intra-chunk contribution: Toeplitz-block matmuls (TD diag block, TU upper)
  - inter-chunk contribution: rank-128 state S (Re/Im of 64 complex modes),
    updated per chunk as S' = MT.T S + AA.T u_chunk, applied as VV.T S
  - D*u folded onto the Toeplitz diagonal
All matmuls in float32r (fp32 with 12-bit-truncated mantissa): full PE speed,
and the HW matmul is exact for pre-rounded inputs (error == input rounding,
~1.2e-4 relative).

Sharding: data-parallel over batch; each of 8 cores gets 512 sequences
(time-major 4096x512 shard).  SSM params are tiny and replicated as seven
128x128 constant matrices computed on host in float64.
"""

import os
import sys

for _p in ("/opt/trn_rl_repo",):
    if _p not in sys.path and os.path.isdir(_p):
        sys.path.append(_p)

import numpy as np

EPS = 1e-7          # complex_softmax eps
B, L, N = 4096, 4096, 64
N_CORES = 8
BC = B // N_CORES   # 512 sequences per core
C = 512             # timesteps per chunk
NB = L // C         # chunks
P = 128             # partitions
H = C // P          # 128-row tiles per chunk

_CNAMES = tuple(f"T{k}" for k in range(H)) + tuple(f"AA{g}" for g in range(H)) \
    + tuple(f"VV{h}" for h in range(H)) + ("MT",)

_PROG = None        # compiled Bass program, built once per process


def _round_fp32r(x: np.ndarray) -> np.ndarray:
    """Round fp32 array to fp32r: keep 11 mantissa bits (round-half-even)."""
    b = np.ascontiguousarray(x, dtype=np.float32).view(np.uint32)
    low = b & np.uint32(0xFFF)
    hi = b & np.uint32(0xFFFFF000)
    half = np.uint32(0x800)
    rnd = (low > half) | ((low == half) & (((hi >> np.uint32(12)) & np.uint32(1)) == 1))
    out = hi + np.where(rnd, np.uint32(0x1000), np.uint32(0))
    return out.view(np.float32)


def _constants(Lambda_re, Lambda_im, W, D, log_step):
    """Seven 128x128 fp32r matrices, computed in float64 on host."""
    step = float(np.exp(np.float64(log_step[0])))
    Lam = Lambda_re.astype(np.float64) + 1j * Lambda_im.astype(np.float64)   # (N,)
    Wc = W[0, :, 0].astype(np.float64) + 1j * W[0, :, 1].astype(np.float64)  # (N,)
    s = np.arange(C + 1, dtype=np.float64)
    pows = np.exp(np.outer(s, step * Lam))                                   # (C+1, N)
    Gamma = pows[C]
    sl = np.arange(L, dtype=np.float64)
    Sigma = np.exp(np.outer(sl, step * Lam)).sum(axis=0)                     # (N,)
    wt = (Wc / Lam) * np.conj(Sigma) / (Sigma * np.conj(Sigma) + EPS)
    K = (pows[:C] * wt[None, :]).sum(axis=1).real                            # (C,)

    idx = np.arange(P)
    qp = idx[None, :] - idx[:, None]                                         # q - p
    mats = {}
    for k in range(H):                                                       # Toeplitz blocks
        if k == 0:
            T = np.where(qp >= 0, K[np.clip(qp, 0, C - 1)], 0.0)
            T = T + np.eye(P) * np.float64(D[0])                             # fold D*u
        else:
            T = K[qp + k * P]
        mats[f"T{k}"] = T
    AP_ = pows[C - 1 - np.arange(C)]                                         # (C, N) r^{C-1-p}
    AA = np.concatenate([AP_.real, AP_.imag], axis=1)                        # (C, 128)
    for g in range(H):
        mats[f"AA{g}"] = AA[g * P:(g + 1) * P]
    MT = np.zeros((P, P), dtype=np.float64)
    n = np.arange(N)
    MT[n, n] = Gamma.real
    MT[64 + n, n] = -Gamma.imag
    MT[n, 64 + n] = Gamma.imag
    MT[64 + n, 64 + n] = Gamma.real
    mats["MT"] = MT
    Vq = pows[1:C + 1] * wt[None, :]                                         # (C, N) wt*r^{q+1}
    VV = np.concatenate([Vq.real.T, -Vq.imag.T], axis=0)                     # (128, C)
    for h in range(H):
        mats[f"VV{h}"] = VV[:, h * P:(h + 1) * P]
    packed = np.concatenate([mats[name] for name in _CNAMES], axis=1)
    return _round_fp32r(packed.astype(np.float32))


def _build():
    import concourse.tile as tile
    from concourse import bacc, mybir
    from contextlib import ExitStack

    f32r, f32 = mybir.dt.float32r, mybir.dt.float32
    nc = bacc.Bacc("TRN2", target_bir_lowering=False, debug=False,
                   num_devices=N_CORES)
    ut = nc.dram_tensor("ut", [L, BC], f32r, kind="ExternalInput").ap()
    yt = nc.dram_tensor("yt", [L, BC], f32, kind="ExternalOutput").ap()
    ncst = len(_CNAMES)
    cap = nc.dram_tensor("CONST", [P, ncst * P], f32r, kind="ExternalInput").ap()

    with tile.TileContext(nc) as tc, ExitStack() as ctx:
        cpool = ctx.enter_context(tc.tile_pool(name="const", bufs=1))
        upool = ctx.enter_context(tc.tile_pool(name="u", bufs=3 * H))
        spool = ctx.enter_context(tc.tile_pool(name="s", bufs=2))
        ypool = ctx.enter_context(tc.tile_pool(name="y", bufs=2 * H))
        pypool = ctx.enter_context(tc.tile_pool(name="psy", bufs=6, space="PSUM"))
        pspool = ctx.enter_context(tc.tile_pool(name="pss", bufs=2, space="PSUM"))

        cstt = cpool.tile([P, ncst * P], f32r, tag="CONST")
        # split the constant load so the first Toeplitz blocks (needed by the
        # very first matmuls) land before the later-needed VV/MT blocks
        nsplit = 2 * H
        nc.scalar.dma_start(cstt[:, :nsplit * P], cap[:, :nsplit * P])
        nc.scalar.dma_start(cstt[:, nsplit * P:], cap[:, nsplit * P:])
        ct = {name: cstt[:, k * P:(k + 1) * P] for k, name in enumerate(_CNAMES)}

        s_prev = None
        for J in range(NB):
            first, last = (J == 0), (J == NB - 1)
            us = []
            for g in range(H):
                ug = upool.tile([P, BC], f32r, tag="u")
                nc.sync.dma_start(ug[:], ut[J * C + g * P: J * C + (g + 1) * P, :])
                us.append(ug)

            psY = [pypool.tile([P, BC], f32, tag="psy", name=f"psY{J}_{h}")
                   for h in range(H)]
            psS = (pspool.tile([P, BC], f32, tag="pss", name=f"psS{J}")
                   if not last else None)
            # u-dependent matmuls, grouped by stationary weight so the PE can
            # reuse/overlap weight loads
            for k in range(H):
                for g in range(H - k):
                    nc.tensor.matmul(psY[g + k][:], ct[f"T{k}"], us[g][:],
                                     start=(k == 0), stop=(first and g == 0))
            for g in range(H):
                if not last:
                    nc.tensor.matmul(psS[:], ct[f"AA{g}"], us[g][:],
                                     start=(g == 0), stop=(first and g == H - 1))
            # state-dependent matmuls; MT first so the next scan step's input
            # (the psS->SBUF copy) is ready as early as possible
            if not first:
                if not last:
                    nc.tensor.matmul(psS[:], ct["MT"], s_prev[:],
                                     start=False, stop=True)
                for h in range(H):
                    nc.tensor.matmul(psY[h][:], ct[f"VV{h}"], s_prev[:],
                                     start=False, stop=True)
            if not last:
                s_new = spool.tile([P, BC], f32r, tag="s")
                nc.vector.tensor_copy(s_new[:], psS[:])
                s_prev = s_new

            for h in range(H):
                yh = ypool.tile([P, BC], f32, tag="y")
                nc.vector.tensor_copy(yh[:], psY[h][:])
                # split the final chunk's stores across both HWDGE queues to
                # shorten the kernel tail
                eng = nc.sync if (last and h % 2 == 0) else nc.scalar
                eng.dma_start(yt[J * C + h * P: J * C + (h + 1) * P, :], yh[:])

    nc.compile()
    return nc


def _program():
    global _PROG
    if _PROG is None:
        _PROG = _build()
    return _PROG


# Set PROFILE=True before calling kernel() to capture an NTFF profile;
# LAST_EXEC_NS then holds the measured hardware execution time.
PROFILE = False
LAST_EXEC_NS = None
LAST_RESULTS = None


def kernel(u, Lambda_re, Lambda_im, W, D, log_step):
    global LAST_EXEC_NS
    from concourse.bass_utils import run_bass_kernel_spmd

    u = np.asarray(u, dtype=np.float32)
    consts = _constants(np.asarray(Lambda_re), np.asarray(Lambda_im),
                        np.asarray(W), np.asarray(D), np.asarray(log_step))
    nc = _program()

    in_maps = []
    for c in range(N_CORES):
        utc = _round_fp32r(np.ascontiguousarray(u[c * BC:(c + 1) * BC, :].T))
        in_maps.append({"ut": utc, "CONST": consts})

    res = run_bass_kernel_spmd(nc, in_maps, list(range(N_CORES)), trace=PROFILE)
    if PROFILE:
        LAST_EXEC_NS = res.exec_time_ns
        global LAST_RESULTS
        LAST_RESULTS = res

    y = np.empty((B, L), dtype=np.float32)
    for c in range(N_CORES):
        y[c * BC:(c + 1) * BC, :] = res.results[c]["yt"].T
    return y


# revision 25
# speedup vs baseline: 1.1776x; 1.1776x over previous
"""Trainium2 Bass kernel for the DSS (Diagonal State Space) layer.

y = irfft(rfft(u, 2L) * rfft(K, 2L))[:L] + D*u, with K the length-L DSS kernel
derived from (Lambda, W, log_step) via a complex softmax.

Implementation: the FFT convolution is mathematically a causal conv with an
exponentially-structured kernel K[s] = Re(sum_n wt_n * r_n^s).  We evaluate it
as a chunked diagonal-SSM scan on-device:
  - time-major layout, chunks of C=256 timesteps (2 partition tiles of 128)
  - intra-chunk contribution: Toeplitz-block matmuls (TD diag block, TU upper)
  - inter-chunk contribution: rank-128 state S (Re/Im of 64 complex modes),
    updated per chunk as S' = MT.T S + AA.T u_chunk, applied as VV.T S
  - D*u folded onto the Toeplitz diagonal
All matmuls in float32r (fp32 with 12-bit-truncated mantissa): full PE speed,
and the HW matmul is exact for pre-rounded inputs (error == input rounding,
~1.2e-4 relative).

Sharding: data-parallel over batch; each of 8 cores gets 512 sequences
(time-major 4096x512 shard).  SSM params are tiny and replicated as seven
128x128 constant matrices computed on host in float64.
"""

import os
import sys

for _p in ("/opt/trn_rl_repo",):
    if _p not in sys.path and os.path.isdir(_p):
        sys.path.append(_p)

import numpy as np

EPS = 1e-7          # complex_softmax eps
B, L, N = 4096, 4096, 64
N_CORES = 8
BC = B // N_CORES   # 512 sequences per core
C = 256             # timesteps per chunk
NB = L // C         # chunks
P = 128             # partitions
H = C // P          # 128-row tiles per chunk

_CNAMES = tuple(f"T{k}" for k in range(H)) + tuple(f"AA{g}" for g in range(H)) \
    + tuple(f"VV{h}" for h in range(H)) + ("MT",)

_PROG = None        # compiled Bass program, built once per process


def _round_fp32r(x: np.ndarray) -> np.ndarray:
    """Round fp32 array to fp32r: keep 11 mantissa bits (round-half-even)."""
    b = np.ascontiguousarray(x, dtype=np.float32).view(np.uint32)
    low = b & np.uint32(0xFFF)
    hi = b & np.uint32(0xFFFFF000)
    half = np.uint32(0x800)
    rnd = (low > half) | ((low == half) & (((hi >> np.uint32(12)) & np.uint32(1)) == 1))
    out = hi + np.where(rnd, np.uint32(0x1000), np.uint32(0))
    return out.view(np.float32)


def _constants(Lambda_re, Lambda_im, W, D, log_step):
    """Seven 128x128 fp32r matrices, computed in float64 on host."""
    step = float(np.exp(np.float64(log_step[0])))
    Lam = Lambda_re.astype(np.float64) + 1j * Lambda_im.astype(np.float64)   # (N,)
    Wc = W[0, :, 0].astype(np.float64) + 1j * W[0, :, 1].astype(np.float64)  # (N,)
    s = np.arange(C + 1, dtype=np.float64)
    pows = np.exp(np.outer(s, step * Lam))                                   # (C+1, N)
    Gamma = pows[C]
    sl = np.arange(L, dtype=np.float64)
    Sigma = np.exp(np.outer(sl, step * Lam)).sum(axis=0)                     # (N,)
    wt = (Wc / Lam) * np.conj(Sigma) / (Sigma * np.conj(Sigma) + EPS)
    K = (pows[:C] * wt[None, :]).sum(axis=1).real                            # (C,)

    idx = np.arange(P)
    qp = idx[None, :] - idx[:, None]                                         # q - p
    mats = {}
    for k in range(H):                                                       # Toeplitz blocks
        if k == 0:
            T = np.where(qp >= 0, K[np.clip(qp, 0, C - 1)], 0.0)
            T = T + np.eye(P) * np.float64(D[0])                             # fold D*u
        else:
            T = K[qp + k * P]
        mats[f"T{k}"] = T
    AP_ = pows[C - 1 - np.arange(C)]                                         # (C, N) r^{C-1-p}
    AA = np.concatenate([AP_.real, AP_.imag], axis=1)                        # (C, 128)
    for g in range(H):
        mats[f"AA{g}"] = AA[g * P:(g + 1) * P]
    MT = np.zeros((P, P), dtype=np.float64)
    n = np.arange(N)
    MT[n, n] = Gamma.real
    MT[64 + n, n] = -Gamma.imag
    MT[n, 64 + n] = Gamma.imag
    MT[64 + n, 64 + n] = Gamma.real
    mats["MT"] = MT
    Vq = pows[1:C + 1] * wt[None, :]                                         # (C, N) wt*r^{q+1}
    VV = np.concatenate([Vq.real.T, -Vq.imag.T], axis=0)                     # (128, C)
    for h in range(H):
        mats[f"VV{h}"] = VV[:, h * P:(h + 1) * P]
    packed = np.concatenate([mats[name] for name in _CNAMES], axis=1)
    return _round_fp32r(packed.astype(np.float32))


def _build():
    import concourse.tile as tile
    from concourse import bacc, mybir
    from contextlib import ExitStack

    f32r, f32 = mybir.dt.float32r, mybir.dt.float32
    nc = bacc.Bacc("TRN2", target_bir_lowering=False, debug=False,
                   num_devices=N_CORES)
    ut = nc.dram_tensor("ut", [L, BC], f32r, kind="ExternalInput").ap()
    yt = nc.dram_tensor("yt", [L, BC], f32, kind="ExternalOutput").ap()
    ncst = len(_CNAMES)
    cap = nc.dram_tensor("CONST", [P, ncst * P], f32r, kind="ExternalInput").ap()

    with tile.TileContext(nc) as tc, ExitStack() as ctx:
        cpool = ctx.enter_context(tc.tile_pool(name="const", bufs=1))
        upool = ctx.enter_context(tc.tile_pool(name="u", bufs=12))
        spool = ctx.enter_context(tc.tile_pool(name="s", bufs=2))
        ypool = ctx.enter_context(tc.tile_pool(name="y", bufs=8))
        pypool = ctx.enter_context(tc.tile_pool(name="psy", bufs=6, space="PSUM"))
        pspool = ctx.enter_context(tc.tile_pool(name="pss", bufs=2, space="PSUM"))

        # PE warmup: a short burst of trivial matmuls on zeroed SBUF keeps the
        # HAM activity window busy during the DMA preamble, so the first real
        # matmuls run at full clock instead of the cold 1.2 GHz.
        warm = cpool.tile([P, 64], mybir.dt.bfloat16, tag="warm")
        nc.gpsimd.memset(warm[:], 0.0)
        psw = pspool.tile([P, BC], f32, tag="pss", name="psw")
        for _ in range(24):
            nc.tensor.matmul(psw[:64, :64], warm[:, :64], warm[:, :64],
                             start=True, stop=True)

        cstt = cpool.tile([P, ncst * P], f32r, tag="CONST")
        # split the constant load so the first Toeplitz blocks (needed by the
        # very first matmuls) land before the later-needed VV/MT blocks
        nsplit = 2 * H
        nc.scalar.dma_start(cstt[:, :nsplit * P], cap[:, :nsplit * P])
        nc.scalar.dma_start(cstt[:, nsplit * P:], cap[:, nsplit * P:])
        ct = {name: cstt[:, k * P:(k + 1) * P] for k, name in enumerate(_CNAMES)}

        s_prev = None
        for J in range(NB):
            first, last = (J == 0), (J == NB - 1)
            us = []
            for g in range(H):
                ug = upool.tile([P, BC], f32r, tag="u")
                nc.sync.dma_start(ug[:], ut[J * C + g * P: J * C + (g + 1) * P, :])
                us.append(ug)

            psY = [pypool.tile([P, BC], f32, tag="psy", name=f"psY{J}_{h}")
                   for h in range(H)]
            psS = (pspool.tile([P, BC], f32, tag="pss", name=f"psS{J}")
                   if not last else None)
            # u-dependent matmuls, grouped by stationary weight so the PE can
            # reuse/overlap weight loads
            for k in range(H):
                for g in range(H - k):
                    nc.tensor.matmul(psY[g + k][:], ct[f"T{k}"], us[g][:],
                                     start=(k == 0), stop=(first and g == 0))
            for g in range(H):
                if not last:
                    nc.tensor.matmul(psS[:], ct[f"AA{g}"], us[g][:],
                                     start=(g == 0), stop=(first and g == H - 1))
            # state-dependent matmuls; MT first so the next scan step's input
            # (the psS->SBUF copy) is ready as early as possible
            if not first:
                if not last:
                    nc.tensor.matmul(psS[:], ct["MT"], s_prev[:],
                                     start=False, stop=True)
                for h in range(H):
                    nc.tensor.matmul(psY[h][:], ct[f"VV{h}"], s_prev[:],
                                     start=False, stop=True)
            if not last:
                s_new = spool.tile([P, BC], f32r, tag="s")
                nc.vector.tensor_copy(s_new[:], psS[:])
                s_prev = s_new

            for h in range(H):
                yh = ypool.tile([P, BC], f32, tag="y")
                nc.vector.tensor_copy(yh[:], psY[h][:])
                # split the final chunk's stores across both HWDGE queues to
                # shorten the kernel tail
                eng = nc.sync if (last and h % 2 == 0) else nc.scalar
                eng.dma_start(yt[J * C + h * P: J * C + (h + 1) * P, :], yh[:])

    nc.compile()
    return nc


def _program():
    global _PROG
    if _PROG is None:
        _PROG = _build()
    return _PROG


# Set PROFILE=True before calling kernel() to capture an NTFF profile;
# LAST_EXEC_NS then holds the measured hardware execution time.
PROFILE = False
LAST_EXEC_NS = None
LAST_RESULTS = None


def kernel(u, Lambda_re, Lambda_im, W, D, log_step):
    global LAST_EXEC_NS
    from concourse.bass_utils import run_bass_kernel_spmd

    u = np.asarray(u, dtype=np.float32)
    consts = _constants(np.asarray(Lambda_re), np.asarray(Lambda_im),
                        np.asarray(W), np.asarray(D), np.asarray(log_step))
    nc = _program()

    in_maps = []
    for c in range(N_CORES):
        utc = _round_fp32r(np.ascontiguousarray(u[c * BC:(c + 1) * BC, :].T))
        in_maps.append({"ut": utc, "CONST": consts})

    res = run_bass_kernel_spmd(nc, in_maps, list(range(N_CORES)), trace=PROFILE)
    if PROFILE:
        LAST_EXEC_NS = res.exec_time_ns
        global LAST_RESULTS
        LAST_RESULTS = res

    y = np.empty((B, L), dtype=np.float32)
    for c in range(N_CORES):
        y[c * BC:(c + 1) * BC, :] = res.results[c]["yt"].T
    return y


# revision 29
# speedup vs baseline: 1.2628x; 1.0723x over previous
"""Trainium2 Bass kernel for the DSS (Diagonal State Space) layer.

y = irfft(rfft(u, 2L) * rfft(K, 2L))[:L] + D*u, with K the length-L DSS kernel
derived from (Lambda, W, log_step) via a complex softmax.

Implementation: the FFT convolution is mathematically a causal conv with an
exponentially-structured kernel K[s] = Re(sum_n wt_n * r_n^s).  We evaluate it
as a chunked diagonal-SSM scan on-device:
  - time-major layout, chunks of C=256 timesteps (2 partition tiles of 128)
  - intra-chunk contribution: Toeplitz-block matmuls (TD diag block, TU upper)
  - inter-chunk contribution: rank-128 state S (Re/Im of 64 complex modes),
    updated per chunk as S' = MT.T S + AA.T u_chunk, applied as VV.T S
  - D*u folded onto the Toeplitz diagonal
All matmuls in float32r (fp32 with 12-bit-truncated mantissa): full PE speed,
and the HW matmul is exact for pre-rounded inputs (error == input rounding,
~1.2e-4 relative).

Sharding: data-parallel over batch; each of 8 cores gets 512 sequences
(time-major 4096x512 shard).  SSM params are tiny and replicated as seven
128x128 constant matrices computed on host in float64.
"""

import os
import sys

for _p in ("/opt/trn_rl_repo",):
    if _p not in sys.path and os.path.isdir(_p):
        sys.path.append(_p)

import numpy as np
import ml_dtypes
_bf = ml_dtypes.bfloat16

EPS = 1e-7          # complex_softmax eps
B, L, N = 4096, 4096, 64
N_CORES = 8
BC = B // N_CORES   # 512 sequences per core
C = 256             # timesteps per chunk
NB = L // C         # chunks
P = 128             # partitions
H = C // P          # 128-row tiles per chunk

_CNAMES = tuple(f"T{k}" for k in range(H)) + tuple(f"AA{g}" for g in range(H)) \
    + tuple(f"VV{h}" for h in range(H)) + ("MT",)

_PROG = None        # compiled Bass program, built once per process


def _round_fp32r(x: np.ndarray) -> np.ndarray:
    """Round fp32 array to fp32r: keep 11 mantissa bits (round-half-even)."""
    b = np.ascontiguousarray(x, dtype=np.float32).view(np.uint32)
    low = b & np.uint32(0xFFF)
    hi = b & np.uint32(0xFFFFF000)
    half = np.uint32(0x800)
    rnd = (low > half) | ((low == half) & (((hi >> np.uint32(12)) & np.uint32(1)) == 1))
    out = hi + np.where(rnd, np.uint32(0x1000), np.uint32(0))
    return out.view(np.float32)


def _constants(Lambda_re, Lambda_im, W, D, log_step):
    """Seven 128x128 fp32r matrices, computed in float64 on host."""
    step = float(np.exp(np.float64(log_step[0])))
    Lam = Lambda_re.astype(np.float64) + 1j * Lambda_im.astype(np.float64)   # (N,)
    Wc = W[0, :, 0].astype(np.float64) + 1j * W[0, :, 1].astype(np.float64)  # (N,)
    s = np.arange(C + 1, dtype=np.float64)
    pows = np.exp(np.outer(s, step * Lam))                                   # (C+1, N)
    Gamma = pows[C]
    sl = np.arange(L, dtype=np.float64)
    Sigma = np.exp(np.outer(sl, step * Lam)).sum(axis=0)                     # (N,)
    wt = (Wc / Lam) * np.conj(Sigma) / (Sigma * np.conj(Sigma) + EPS)
    K = (pows[:C] * wt[None, :]).sum(axis=1).real                            # (C,)

    idx = np.arange(P)
    qp = idx[None, :] - idx[:, None]                                         # q - p
    mats = {}
    for k in range(H):                                                       # Toeplitz blocks
        if k == 0:
            T = np.where(qp >= 0, K[np.clip(qp, 0, C - 1)], 0.0)
            T = T + np.eye(P) * np.float64(D[0])                             # fold D*u
        else:
            T = K[qp + k * P]
        mats[f"T{k}"] = T
    AP_ = pows[C - 1 - np.arange(C)]                                         # (C, N) r^{C-1-p}
    AA = np.concatenate([AP_.real, AP_.imag], axis=1)                        # (C, 128)
    for g in range(H):
        mats[f"AA{g}"] = AA[g * P:(g + 1) * P]
    MT = np.zeros((P, P), dtype=np.float64)
    n = np.arange(N)
    MT[n, n] = Gamma.real
    MT[64 + n, n] = -Gamma.imag
    MT[n, 64 + n] = Gamma.imag
    MT[64 + n, 64 + n] = Gamma.real
    mats["MT"] = MT
    Vq = pows[1:C + 1] * wt[None, :]                                         # (C, N) wt*r^{q+1}
    VV = np.concatenate([Vq.real.T, -Vq.imag.T], axis=0)                     # (128, C)
    for h in range(H):
        mats[f"VV{h}"] = VV[:, h * P:(h + 1) * P]
    packed = np.concatenate([mats[name] for name in _CNAMES], axis=1)
    import ml_dtypes
    return packed.astype(np.float32).astype(ml_dtypes.bfloat16)


def _build():
    import concourse.tile as tile
    from concourse import bacc, mybir
    from contextlib import ExitStack

    f32r, f32 = mybir.dt.float32r, mybir.dt.float32
    nc = bacc.Bacc("TRN2", target_bir_lowering=False, debug=False,
                   num_devices=N_CORES)
    bf16 = mybir.dt.bfloat16
    ut = nc.dram_tensor("ut", [L, BC], bf16, kind="ExternalInput").ap()
    yt = nc.dram_tensor("yt", [L, BC], f32, kind="ExternalOutput").ap()
    ncst = len(_CNAMES)
    cap = nc.dram_tensor("CONST", [P, ncst * P], bf16, kind="ExternalInput").ap()

    with tile.TileContext(nc) as tc, ExitStack() as ctx:
        cpool = ctx.enter_context(tc.tile_pool(name="const", bufs=1))
        upool = ctx.enter_context(tc.tile_pool(name="u", bufs=12))
        spool = ctx.enter_context(tc.tile_pool(name="s", bufs=2))
        ypool = ctx.enter_context(tc.tile_pool(name="y", bufs=8))
        pypool = ctx.enter_context(tc.tile_pool(name="psy", bufs=6, space="PSUM"))
        pspool = ctx.enter_context(tc.tile_pool(name="pss", bufs=2, space="PSUM"))

        # PE warmup: a short burst of trivial matmuls on zeroed SBUF keeps the
        # HAM activity window busy during the DMA preamble, so the first real
        # matmuls run at full clock instead of the cold 1.2 GHz.
        warm = cpool.tile([P, 64], mybir.dt.bfloat16, tag="warm")
        nc.gpsimd.memset(warm[:], 0.0)
        psw = pspool.tile([P, BC], f32, tag="pss", name="psw")
        for _ in range(24):
            nc.tensor.matmul(psw[:64, :64], warm[:, :64], warm[:, :64],
                             start=True, stop=True)

        cstt = cpool.tile([P, ncst * P], bf16, tag="CONST")
        # split the constant load so the first Toeplitz blocks (needed by the
        # very first matmuls) land before the later-needed VV/MT blocks
        nsplit = 2 * H
        nc.scalar.dma_start(cstt[:, :nsplit * P], cap[:, :nsplit * P])
        nc.scalar.dma_start(cstt[:, nsplit * P:], cap[:, nsplit * P:])
        ct = {name: cstt[:, k * P:(k + 1) * P] for k, name in enumerate(_CNAMES)}

        s_prev = None
        for J in range(NB):
            first, last = (J == 0), (J == NB - 1)
            us = []
            for g in range(H):
                ug = upool.tile([P, BC], bf16, tag="u")
                nc.sync.dma_start(ug[:], ut[J * C + g * P: J * C + (g + 1) * P, :])
                us.append(ug)

            psY = [pypool.tile([P, BC], f32, tag="psy", name=f"psY{J}_{h}")
                   for h in range(H)]
            psS = (pspool.tile([P, BC], f32, tag="pss", name=f"psS{J}")
                   if not last else None)
            # u-dependent matmuls, grouped by stationary weight so the PE can
            # reuse/overlap weight loads
            for k in range(H):
                for g in range(H - k):
                    nc.tensor.matmul(psY[g + k][:], ct[f"T{k}"], us[g][:],
                                     start=(k == 0), stop=(first and g == 0))
            for g in range(H):
                if not last:
                    nc.tensor.matmul(psS[:], ct[f"AA{g}"], us[g][:],
                                     start=(g == 0), stop=(first and g == H - 1))
            # state-dependent matmuls; MT first so the next scan step's input
            # (the psS->SBUF copy) is ready as early as possible
            if not first:
                if not last:
                    nc.tensor.matmul(psS[:], ct["MT"], s_prev[:],
                                     start=False, stop=True)
                for h in range(H):
                    nc.tensor.matmul(psY[h][:], ct[f"VV{h}"], s_prev[:],
                                     start=False, stop=True)
            if not last:
                s_new = spool.tile([P, BC], bf16, tag="s")
                nc.vector.tensor_copy(s_new[:], psS[:])
                s_prev = s_new

            for h in range(H):
                yh = ypool.tile([P, BC], f32, tag="y")
                nc.vector.tensor_copy(yh[:], psY[h][:])
                # split the final chunk's stores across both HWDGE queues to
                # shorten the kernel tail
                eng = nc.sync if (last and h % 2 == 0) else nc.scalar
                eng.dma_start(yt[J * C + h * P: J * C + (h + 1) * P, :], yh[:])

    nc.compile()
    return nc


def _program():
    global _PROG
    if _PROG is None:
        _PROG = _build()
    return _PROG


# Set PROFILE=True before calling kernel() to capture an NTFF profile;
# LAST_EXEC_NS then holds the measured hardware execution time.
PROFILE = False
LAST_EXEC_NS = None
LAST_RESULTS = None


def kernel(u, Lambda_re, Lambda_im, W, D, log_step):
    global LAST_EXEC_NS
    from concourse.bass_utils import run_bass_kernel_spmd

    u = np.asarray(u, dtype=np.float32)
    consts = _constants(np.asarray(Lambda_re), np.asarray(Lambda_im),
                        np.asarray(W), np.asarray(D), np.asarray(log_step))
    nc = _program()

    in_maps = []
    for c in range(N_CORES):
        utc = np.ascontiguousarray(u[c * BC:(c + 1) * BC, :].T).astype(_bf)
        in_maps.append({"ut": utc, "CONST": consts})

    res = run_bass_kernel_spmd(nc, in_maps, list(range(N_CORES)), trace=PROFILE)
    if PROFILE:
        LAST_EXEC_NS = res.exec_time_ns
        global LAST_RESULTS
        LAST_RESULTS = res

    y = np.empty((B, L), dtype=np.float32)
    for c in range(N_CORES):
        y[c * BC:(c + 1) * BC, :] = res.results[c]["yt"].T
    return y
